# revision 4
# baseline (speedup 1.0000x reference)
"""Trainium2 Bass kernel for nn_MemorizingTransformer.

Sharding: 8 cores = 4 batches x 2 head-groups (4 heads each). Each core runs
an identical SPMD program on its (batch, head-group) slice and produces a
partial (n, dim) output; host sums the two head-group partials per batch.

Per-core algorithm (slab-streamed, slab = 128 query rows of one head):
  - q/kv projections + l2norm on device (f32r matmuls)
  - S_mem = q_hat @ mem_k_hat^T in PSUM (f32r, rne11-rounded inputs)
  - exp via ACT directly from PSUM -> bf16 e
  - top-32 threshold per row: chunk top-8 (vector.max per 256-chunk) ->
    finalist rounds (max/match_replace) -> 32nd value as threshold
  - masked P = (e >= thr) * e in one fused scalar_tensor_tensor pass
  - P^T via DMA transpose; PV matmul accumulates mem + local parts and a
    ones-column computes the softmax denominator in the same PSUM tile
  - local (xl+causal) attention handled the same way with exp+zeroed diag
  - output projection in f32r
"""

import os
import sys

import numpy as np

sys.path.insert(0, "/opt/trn_rl_repo")

import concourse.bass as bass  # noqa: E402
import concourse.mybir as mybir  # noqa: E402
import concourse.tile as tile  # noqa: E402
from concourse import bacc  # noqa: E402
from concourse.bass_utils import run_bass_kernel_spmd  # noqa: E402
from concourse.masks import make_identity  # noqa: E402

dt = mybir.dt
AF = mybir.ActivationFunctionType
ALU = mybir.AluOpType
AX = mybir.AxisListType

HEADS = 8
DH = 64
KNN = 32


class Geom:
    def __init__(self, n=1024, dim=512, xl=512, m=4096, hpc=4):
        self.N = n
        self.DIM = dim
        self.XL = xl
        self.M = m
        self.HPC = hpc              # heads per core
        self.J = xl + n             # local+xl keys
        self.NT = n // 128          # i-tiles
        self.MT = m // 128          # 128-wide m chunks
        self.MC512 = m // 512       # 512-wide matmul chunks
        self.KC = dim // 128        # contraction chunks for projections
        self.QW = hpc * DH          # q width per core
        self.W = self.QW + 2 * DH   # qkv projection width
        self.JT = self.J // 128     # 128-wide local chunks
        self.TOPC = 256             # chunk width for candidate top-8
        self.NCH = m // self.TOPC   # number of candidate chunks


def build_program(g: Geom) -> bacc.Bacc:
    nc = bacc.Bacc("TRN2", target_bir_lowering=False, num_devices=8)

    xT = nc.dram_tensor("xT", [g.DIM, g.N], dt.float32r, kind="ExternalInput")
    Wcat = nc.dram_tensor("Wcat", [g.DIM, g.W], dt.float32r, kind="ExternalInput")
    kxlT = nc.dram_tensor("kxlT", [DH, g.XL], dt.float32r, kind="ExternalInput")
    vxl = nc.dram_tensor("vxl", [g.XL, DH], dt.float32, kind="ExternalInput")
    mk = nc.dram_tensor("mk", [g.M, DH], dt.float32, kind="ExternalInput")
    mv = nc.dram_tensor("mv", [g.M, DH], dt.float32, kind="ExternalInput")
    WoutT = nc.dram_tensor("WoutT", [g.QW, g.DIM], dt.float32r, kind="ExternalInput")
    scales = nc.dram_tensor("scales", [128, g.HPC], dt.float32, kind="ExternalInput")
    out = nc.dram_tensor("out_part", [g.N, g.DIM], dt.float32, kind="ExternalOutput")

    with tile.TileContext(nc) as tc:
        with (
            tc.tile_pool(name="const", bufs=1) as cpool,
            tc.tile_pool(name="ld", bufs=3) as ldpool,
            tc.tile_pool(name="slab", bufs=2) as spool,
            tc.tile_pool(name="small", bufs=3) as smpool,
            tc.tile_pool(name="ps512", bufs=2, space="PSUM") as ps512,
            tc.tile_pool(name="psloc", bufs=3, space="PSUM") as psloc,
            tc.tile_pool(name="pso", bufs=1, space="PSUM") as pso,
            tc.tile_pool(name="pstr", bufs=2, space="PSUM") as pstr,
        ):
            # ---------------- Phase A: loads + prep ----------------
            ident = cpool.tile([128, 128], dt.float32)
            make_identity(nc, ident[:])

            xt_t = cpool.tile([128, g.KC, g.N], dt.float32r, tag="xt")
            nc.sync.dma_start(xt_t[:], xT[:].rearrange("(c p) n -> p c n", p=128))
            wc_t = cpool.tile([128, g.KC, g.W], dt.float32r, tag="wc")
            nc.sync.dma_start(wc_t[:], Wcat[:].rearrange("(c p) n -> p c n", p=128))
            wo_t = cpool.tile([128, g.QW // 128, g.DIM], dt.float32r, tag="wo")
            nc.sync.dma_start(wo_t[:], WoutT[:].rearrange("(c p) n -> p c n", p=128))
            s_t = cpool.tile([128, g.HPC], dt.float32, tag="sc")
            nc.sync.dma_start(s_t[:], scales[:])

            kallT = cpool.tile([64, g.J], dt.float32r, tag="kallT")
            nc.sync.dma_start(kallT[:, 0 : g.XL], kxlT[:])

            # v_all (+ones col) in bf16: [128, JT, 65]
            vall = cpool.tile([128, g.JT, DH + 1], dt.bfloat16, tag="vall")
            nc.vector.memset(vall[:], 0.0)
            for c in range(g.XL // 128):
                vt = ldpool.tile([128, DH], dt.float32, tag="vload")
                nc.sync.dma_start(vt[:], vxl[c * 128 : (c + 1) * 128, :])
                nc.vector.tensor_copy(vall[:, c, 0:DH], vt[:])

            # memory values (+ones) bf16: [128, MT, 65]
            mvall = cpool.tile([128, g.MT, DH + 1], dt.bfloat16, tag="mvall")
            for c in range(g.MT):
                vt = ldpool.tile([128, DH], dt.float32, tag="vload")
                nc.sync.dma_start(vt[:], mv[c * 128 : (c + 1) * 128, :])
                nc.vector.tensor_copy(mvall[:, c, 0:DH], vt[:])
                nc.vector.memset(mvall[:, c, DH : DH + 1], 1.0)
            for c in range(g.JT):
                nc.vector.memset(vall[:, c, DH : DH + 1], 1.0)

            # normalize memory keys, write transposed f32r [64, M]
            mknT = cpool.tile([64, g.M], dt.float32r, tag="mknT")
            for c in range(g.MT):
                kt = ldpool.tile([128, DH], dt.float32, tag="kload")
                nc.sync.dma_start(kt[:], mk[c * 128 : (c + 1) * 128, :])
                sq = smpool.tile([128, DH], dt.float32, tag="sq")
                nc.vector.tensor_tensor(sq[:], kt[:], kt[:], op=ALU.mult)
                ss = smpool.tile([128, 1], dt.float32, tag="ss")
                nc.vector.tensor_reduce(ss[:], sq[:], axis=AX.X, op=ALU.add)
                rr = smpool.tile([128, 1], dt.float32, tag="rr")
                nc.vector.reciprocal(rr[:], ss[:])
                rs = smpool.tile([128, 1], dt.float32, tag="rs")
                nc.scalar.activation(rs[:], rr[:], AF.Sqrt)
                knrm = ldpool.tile([128, DH], dt.float32, tag="knrm")
                nc.vector.tensor_scalar_mul(knrm[:], kt[:], rs[:])
                ptr = pstr.tile([64, 128], dt.float32, tag="ptr")
                nc.tensor.transpose(ptr[:], knrm[:], ident[:])
                nc.scalar.copy(mknT[:, c * 128 : (c + 1) * 128], ptr[:])

            # projections + q/k norms per i-tile
            qnT = cpool.tile([64, g.HPC * g.N], dt.float32r, tag="qnT")
            acc = cpool.tile([128, g.NT, g.QW], dt.float32, tag="acc")
            for it in range(g.NT):
                pq = ps512.tile([128, g.W], dt.float32, tag="ps512")
                for kc in range(g.KC):
                    nc.tensor.matmul(
                        pq[:],
                        xt_t[:, kc, it * 128 : (it + 1) * 128],
                        wc_t[:, kc, :],
                        start=(kc == 0),
                        stop=(kc == g.KC - 1),
                    )
                qkv = ldpool.tile([128, g.W], dt.float32, tag="qkv")
                nc.scalar.copy(qkv[:], pq[:])
                nh = g.HPC + 1
                sq = smpool.tile([128, nh * DH], dt.float32, tag="sqq")
                nc.vector.tensor_tensor(
                    sq[:], qkv[:, 0 : nh * DH], qkv[:, 0 : nh * DH], op=ALU.mult
                )
                ss = smpool.tile([128, nh], dt.float32, tag="ssq")
                nc.vector.tensor_reduce(
                    ss[:], sq[:].rearrange("p (h d) -> p h d", d=DH), axis=AX.X, op=ALU.add
                )
                rr = smpool.tile([128, nh], dt.float32, tag="rrq")
                nc.vector.reciprocal(rr[:], ss[:])
                rs = smpool.tile([128, nh], dt.float32, tag="rsq")
                nc.scalar.activation(rs[:], rr[:], AF.Sqrt)
                qs = smpool.tile([128, g.HPC], dt.float32, tag="qs")
                nc.vector.tensor_tensor(qs[:], rs[:, 0 : g.HPC], s_t[:], op=ALU.mult)
                qsc = ldpool.tile([128, g.HPC * DH], dt.float32, tag="qsc")
                for h in range(g.HPC):
                    nc.vector.tensor_scalar_mul(
                        qsc[:, h * DH : (h + 1) * DH],
                        qkv[:, h * DH : (h + 1) * DH],
                        qs[:, h : h + 1],
                    )
                knl = ldpool.tile([128, DH], dt.float32, tag="knl")
                nc.vector.tensor_scalar_mul(
                    knl[:], qkv[:, g.HPC * DH : (g.HPC + 1) * DH], rs[:, g.HPC : g.HPC + 1]
                )
                for h in range(g.HPC):
                    ptr = pstr.tile([64, 128], dt.float32, tag="ptr")
                    nc.tensor.transpose(ptr[:], qsc[:, h * DH : (h + 1) * DH], ident[:])
                    nc.scalar.copy(
                        qnT[:, h * g.N + it * 128 : h * g.N + (it + 1) * 128], ptr[:]
                    )
                ptr = pstr.tile([64, 128], dt.float32, tag="ptr")
                nc.tensor.transpose(ptr[:], knl[:], ident[:])
                nc.scalar.copy(
                    kallT[:, g.XL + it * 128 : g.XL + (it + 1) * 128], ptr[:]
                )
                vt = ldpool.tile([128, DH], dt.float32, tag="vload")
                nc.vector.tensor_copy(vt[:], qkv[:, (g.HPC + 1) * DH : g.W])
                nc.vector.tensor_copy(
                    vall[:, g.XL // 128 + it, 0:DH], vt[:]
                )

            # ---------------- Phase B: slabs ----------------
            for h in range(g.HPC):
                for it in range(g.NT):
                    lh = qnT[:, h * g.N + it * 128 : h * g.N + (it + 1) * 128]

                    # local sims first: needed for the row-max exp bias
                    wvis = g.XL + (it + 1) * 128
                    nch = (wvis + 511) // 512
                    psls = []
                    cmax = smpool.tile([128, 4], dt.float32, tag="cmax")
                    for lc in range(nch):
                        wck = min(512, g.J - lc * 512)
                        psL = psloc.tile([128, 512], dt.float32, tag="psloc")
                        psls.append((psL, wck))
                        nc.tensor.matmul(
                            psL[:, 0:wck], lh,
                            kallT[:, lc * 512 : lc * 512 + wck],
                            start=True, stop=True,
                        )
                        nc.vector.tensor_reduce(
                            cmax[:, lc : lc + 1], psL[:, 0:wck], axis=AX.X, op=ALU.max
                        )
                    # mem logits are bounded by the scale (|q_hat*s| * |k_hat| <= s)
                    nc.vector.tensor_copy(cmax[:, nch : nch + 1], s_t[:, h : h + 1])
                    mrow = smpool.tile([128, 1], dt.float32, tag="mrow")
                    nc.vector.tensor_reduce(
                        mrow[:], cmax[:, 0 : nch + 1], axis=AX.X, op=ALU.max
                    )
                    negm = smpool.tile([128, 1], dt.float32, tag="negm")
                    nc.vector.tensor_scalar(
                        negm[:], mrow[:], -1.0, None, op0=ALU.mult
                    )
                    el = spool.tile([128, g.J], dt.bfloat16, tag="el")
                    for lc in range(nch):
                        psL, wck = psls[lc]
                        nc.scalar.activation(
                            el[:, lc * 512 : lc * 512 + wck], psL[:, 0:wck],
                            AF.Exp, bias=negm[:],
                        )

                    # memory sims -> exp with same bias
                    e = spool.tile([128, g.M], dt.bfloat16, tag="e")
                    for mc in range(g.MC512):
                        psS = ps512.tile([128, 512], dt.float32, tag="ps512")
                        nc.tensor.matmul(
                            psS[:], lh, mknT[:, mc * 512 : (mc + 1) * 512],
                            start=True, stop=True,
                        )
                        nc.scalar.activation(
                            e[:, mc * 512 : (mc + 1) * 512], psS[:], AF.Exp,
                            bias=negm[:],
                        )
                    # candidate chunk top-8s
                    cand = smpool.tile([128, 8 * g.NCH], dt.bfloat16, tag="cand")
                    for c in range(g.NCH):
                        nc.vector.max(
                            out=cand[:, c * 8 : (c + 1) * 8],
                            in_=e[:, c * g.TOPC : (c + 1) * g.TOPC],
                        )
                    # finalist rounds -> 32nd largest
                    w8 = smpool.tile([128, 8], dt.bfloat16, tag="w8")
                    for r in range(KNN // 8):
                        nc.vector.max(out=w8[:], in_=cand[:])
                        if r < KNN // 8 - 1:
                            nc.vector.match_replace(
                                out=cand[:], in_to_replace=w8[:], in_values=cand[:],
                                imm_value=0.0,
                            )
                    thr = smpool.tile([128, 1], dt.float32, tag="thr")
                    nc.vector.tensor_copy(thr[:], w8[:, 7:8])
                    # fused mask+mult
                    p = spool.tile([128, g.M], dt.bfloat16, tag="p")
                    nc.vector.scalar_tensor_tensor(
                        p[:], e[:], thr[:], e[:], op0=ALU.is_ge, op1=ALU.mult
                    )
                    # transpose P via DMA
                    pT = spool.tile([128, g.MT, 128], dt.bfloat16, tag="pT")
                    nc.sync.dma_start_transpose(pT[:], p[:])

                    dcol = g.XL + it * 128
                    nc.gpsimd.affine_select(
                        out=el[:, dcol : dcol + 128],
                        in_=el[:, dcol : dcol + 128],
                        compare_op=ALU.is_ge,
                        fill=0.0,
                        base=0,
                        pattern=[[-1, 128]],
                        channel_multiplier=1,
                    )
                    njc = g.XL // 128 + it + 1
                    elT = spool.tile([128, g.JT, 128], dt.bfloat16, tag="elT")
                    nc.sync.dma_start_transpose(
                        elT[:, 0:njc, :], el[:, 0 : njc * 128]
                    )

                    # PV: mem + local accumulate into one psum (+ ones col)
                    po = pso.tile([128, DH + 1], dt.float32, tag="pso")
                    for mc in range(g.MT):
                        nc.tensor.matmul(
                            po[:], pT[:, mc, :], mvall[:, mc, :],
                            start=(mc == 0), stop=False,
                        )
                    for jc in range(njc):
                        nc.tensor.matmul(
                            po[:], elT[:, jc, :], vall[:, jc, :],
                            start=False, stop=(jc == njc - 1),
                        )
                    rec = smpool.tile([128, 1], dt.float32, tag="rec")
                    nc.vector.reciprocal(rec[:], po[:, DH : DH + 1])
                    nc.vector.tensor_scalar_mul(
                        acc[:, it, h * DH : (h + 1) * DH], po[:, 0:DH], rec[:]
                    )

            # ---------------- Phase C: output projection ----------------
            for it in range(g.NT):
                pout = ps512.tile([128, g.DIM], dt.float32, tag="ps512")
                for kc in range(g.QW // 128):
                    ptr = pstr.tile([128, 128], dt.float32, tag="ptr")
                    nc.tensor.transpose(
                        ptr[:], acc[:, it, kc * 128 : (kc + 1) * 128], ident[:]
                    )
                    accT = ldpool.tile([128, 128], dt.float32r, tag="accT")
                    nc.scalar.copy(accT[:], ptr[:])
                    nc.tensor.matmul(
                        pout[:], accT[:], wo_t[:, kc, :],
                        start=(kc == 0), stop=(kc == g.QW // 128 - 1),
                    )
                osb = ldpool.tile([128, g.DIM], dt.float32, tag="osb")
                nc.scalar.copy(osb[:], pout[:])
                nc.sync.dma_start(out[it * 128 : (it + 1) * 128, :], osb[:])

    nc.compile()
    return nc


_programs = {}


def _get_program(g: Geom):
    key = (g.N, g.DIM, g.XL, g.M, g.HPC)
    if key not in _programs:
        _programs[key] = build_program(g)
    return _programs[key]


def make_in_maps(x, xl_memory, knn_db, Wq, Wkv, Wout, scale_param, g: Geom):
    b = x.shape[0]
    scales_full = np.exp(np.asarray(scale_param, dtype=np.float32).reshape(-1))
    n_hg = HEADS // g.HPC
    in_maps = []
    for core in range(8):
        bi = core // n_hg
        hg = core % n_hg
        h0 = hg * g.HPC
        wq_hg = Wq[:, h0 * DH : (h0 + g.HPC) * DH]
        in_maps.append(
            {
                "xT": np.ascontiguousarray(x[bi].T),
                "Wcat": np.ascontiguousarray(
                    np.concatenate([wq_hg, Wkv], axis=1)
                ),
                "kxlT": np.ascontiguousarray(xl_memory[bi, :, 0, :].T),
                "vxl": np.ascontiguousarray(xl_memory[bi, :, 1, :]),
                "mk": np.ascontiguousarray(knn_db[bi, :, 0, :]),
                "mv": np.ascontiguousarray(knn_db[bi, :, 1, :]),
                "WoutT": np.ascontiguousarray(
                    Wout[h0 * DH : (h0 + g.HPC) * DH, :]
                ),
                "scales": np.tile(
                    scales_full[h0 : h0 + g.HPC][None, :], (128, 1)
                ).astype(np.float32),
            }
        )
    return in_maps


def kernel(x, xl_memory, knn_db, Wq, Wkv, Wout, scale_param, **run_kwargs):
    x = np.asarray(x, dtype=np.float32)
    xl_memory = np.asarray(xl_memory, dtype=np.float32)
    knn_db = np.asarray(knn_db, dtype=np.float32)
    Wq = np.asarray(Wq, dtype=np.float32)
    Wkv = np.asarray(Wkv, dtype=np.float32)
    Wout = np.asarray(Wout, dtype=np.float32)
    b, n, dim = x.shape
    g = Geom(n=n, dim=dim, xl=xl_memory.shape[1], m=knn_db.shape[1], hpc=HEADS // 2)
    nc = _get_program(g)
    in_maps = make_in_maps(x, xl_memory, knn_db, Wq, Wkv, Wout, scale_param, g)
    res = run_bass_kernel_spmd(nc, in_maps, core_ids=list(range(8)), **run_kwargs)
    kernel.last_run = res
    out = np.zeros((b, n, dim), dtype=np.float32)
    n_hg = HEADS // g.HPC
    for core in range(8):
        bi = core // n_hg
        out[bi] += res.results[core]["out_part"]
    return out
